# revision 8
# baseline (speedup 1.0000x reference)
"""Bilinear decoder kernel for Trainium2 (8 NeuronCores, SPMD).

Computes score[b] = head[b]^T @ relation_matrices[relation_ids[b]] @ tail[b]
for b in [0, 4096).

Strategy (relation-grouped sharding, bf16, fused per-block bundles):
  Host: group samples by relation id into blocks of <=256 rows (each block
  uses exactly one relation matrix), pad the block list so all 8 cores run
  the same block count (SPMD: one program). Per block, pack ONE bf16 bundle
  [128, 1536]: matrix in matmul layout | headT (transposed, per
  subtile x contraction-half) | tail (per subtile). One DMA per block:
  block 1 on the ACT ring (nc.scalar), blocks 0,2,3,... sequential on the
  SP ring (nc.sync) — staggered landing (~1.8us cadence) feeds the serial
  DVE chain without idle gaps.
  Device (per block): 4 accumulated bf16 matmuls
  psum[128b, 2*256j] += headT[128i, 128b].T @ M[128i, 256j], then one DVE
  multiply (psum * tail -> bf16 scratch) and one segmented row-sum
  giving the per-row scores.
  Host: scatter scores back through the sort permutation.

bf16 inputs, fp32 PSUM accumulate + fp32 reduce: measured absmax-relative
error 3.0e-3 on the full problem (gate 2e-2).

Variants measured on HW and rejected (exec_time max-core, vs this
kernel's ~21.9-22.2us from baseline 26.1-27.3us):
  - tensor_tensor_reduce (fused multiply+reduce): crashes the DVE on real
    HW (NRT_EXEC_UNIT_UNRECOVERABLE) despite passing CoreSim.
  - all 4 bundle DMAs sequential on one ring: ~0.7us completion bubble
    between same-ring DMAs -> last block lands later (23.0us).
  - blk0 + fused-middle + blk-last DMA split: the big fused stream starves
    block 0 (22.3us).
  - PE warm-up matmul chain (HAM clock gate): warms early blocks but adds
    ~40ns/instruction to the semaphore preamble; net loss (22.8us).
  - reduces on the ACT engine via activation(Copy, accum_out): ~600ns per
    subtile on ACT + 1.3us ACT_TABLE_LOAD; net loss (23.2-23.8us).
  - psum->bf16 via ACT then all-bf16 DVE ops (2x_1p mode): handoff latency
    exceeds the 2x TT gain (23.8us).
  - 2-bank PSUM tile with pair-fused DVE reduce: tile-granularity deps
    serialize the pair (25.5us).
  - fp8 matrices: absmax-rel 4.3e-2 > 2e-2 gate. DoubleRow: fp8-only.
"""

import numpy as np
import ml_dtypes

P = 128
DIM = 256
BLOCK = 256  # rows per block (one relation per block)
NCORES = 8
BF = ml_dtypes.bfloat16
W = 6 * DIM  # bundle columns: 512 matrix | 512 headT | 512 tail

_prog_cache = {}

# test-harness knobs: set TRACE=True before calling kernel() to capture an
# NTFF profile; the BassKernelResults lands in LAST_RESULT.
TRACE = False
LAST_RESULT = None


def _build(n_blocks):
    import concourse.bacc as bacc
    import concourse.mybir as mybir
    import concourse.tile as tile

    f32 = mybir.dt.float32
    bf16 = mybir.dt.bfloat16

    nc = bacc.Bacc("TRN2", target_bir_lowering=False)
    blk_in = nc.dram_tensor("blk", [n_blocks, P, W], bf16, kind="ExternalInput")
    out = nc.dram_tensor("out", [P, 2 * n_blocks], f32, kind="ExternalOutput")

    with tile.TileContext(nc) as tc:
        with (
            tc.tile_pool(name="blk", bufs=max(n_blocks, 2)) as blk_pool,
            tc.tile_pool(name="scr", bufs=2) as scr_pool,
            tc.tile_pool(name="o", bufs=1) as o_pool,
            tc.tile_pool(name="psum", bufs=min(n_blocks, 4), space="PSUM") as psum_pool,
        ):
            # ring split (HW-measured): blocks 0+1 stream concurrently on the
            # two rings and land first; the remaining blocks all go on the SP
            # ring behind block 0 so each streams alone at FULL rate and
            # lands staggered (~1.8us cadence incl the per-DMA completion
            # bubble) instead of the last pair landing together late — this
            # feeds the serial DVE chain without idle gaps.
            # per block: (mh_tile, mh_off, t_tile, t_off). Block 1 on the
            # ACT ring; blocks 0,2,3,... sequential on the SP ring so the
            # later blocks land staggered (~1.8us cadence incl the per-DMA
            # completion bubble) and feed the serial DVE chain gaplessly.
            # Splitting a bundle into smaller staggered DMAs regresses
            # ~2.5-3us whether targeting one tile or two (small HWDGE
            # transfers are descriptor-inefficient).
            slots = []
            for u in range(n_blocks):
                t = blk_pool.tile([P, W], bf16, tag="blk")
                eng = nc.scalar if u == 1 else nc.sync
                eng.dma_start(out=t[:], in_=blk_in[u, :, :])
                slots.append((t, 0, t, 4 * DIM))
            out_sb = o_pool.tile([P, 2 * n_blocks], f32)

            for u in range(n_blocks):
                mh, mo, tt, to = slots[u]
                ps = psum_pool.tile([P, 2 * DIM], f32)
                for s in range(2):
                    for c in range(2):
                        nc.tensor.matmul(
                            out=ps[:, s * DIM : (s + 1) * DIM],
                            lhsT=mh[:, mo + 2 * DIM + (s * 2 + c) * P : mo + 2 * DIM + (s * 2 + c + 1) * P],
                            rhs=mh[:, mo + c * DIM : mo + (c + 1) * DIM],
                            start=(c == 0),
                            stop=(c == 1),
                        )
                scr = scr_pool.tile([P, 2 * DIM], bf16, tag="scr")
                nc.vector.tensor_tensor(
                    out=scr[:],
                    in0=ps[:],
                    in1=tt[:, to : to + 2 * DIM],
                    op=mybir.AluOpType.mult,
                )
                nc.vector.reduce_sum(
                    out=out_sb[:, u * 2 : u * 2 + 2],
                    in_=scr[:].rearrange("p (s j) -> p s j", s=2),
                    axis=mybir.AxisListType.X,
                )

            nc.scalar.dma_start(out=out[:, :], in_=out_sb[:])

    nc.compile()
    return nc


def _plan(ids, R):
    """Group sample indices by relation, chunk to <=BLOCK-row blocks, pad to
    a uniform per-core block count. Returns (n_blocks, blocks) where blocks
    is a list of (relation, sample_indices) of length n_blocks * NCORES."""
    blocks = []
    for r in range(R):
        idxs = np.nonzero(ids == r)[0]
        for s in range(0, len(idxs), BLOCK):
            blocks.append((r, idxs[s : s + BLOCK]))
    if not blocks:
        blocks.append((0, np.empty(0, np.int64)))
    n_blocks = -(-len(blocks) // NCORES)
    empty = np.empty(0, np.int64)
    while len(blocks) < n_blocks * NCORES:
        blocks.append((0, empty))
    return n_blocks, blocks


def _core_inputs(head, tail, mstack, blocks, k, n_blocks):
    """mstack: [R, P, 2*DIM] bf16 per-relation matrices in matmul layout.
    Bundle layout per block u (bf16 [P, 1536]):
      cols [0,512):      m[p, c*256+j]          = M_r[c*128+p, j]
      cols [512,1024):   h[p, (s*2+c)*128 + b]  = head[sub_s[b], c*128+p]
      cols [1024,1536):  t[p, s*256+j]          = tail[sub_s[p], j]
    """
    cblocks = blocks[k * n_blocks : (k + 1) * n_blocks]
    blk = np.zeros((n_blocks, P, W), BF)
    for u, (r, samp) in enumerate(cblocks):
        blk[u, :, 0 : 2 * DIM] = mstack[r]
        for s in range(2):
            sub = samp[s * P : (s + 1) * P]
            nb = len(sub)
            if nb == 0:
                continue
            ht = head[sub].astype(BF).T  # [DIM, nb]
            c0 = 2 * DIM + (s * 2) * P
            blk[u, :, c0 : c0 + nb] = ht[:P, :]
            blk[u, :, c0 + P : c0 + P + nb] = ht[P:, :]
            blk[u, :nb, 4 * DIM + s * DIM : 4 * DIM + (s + 1) * DIM] = tail[sub].astype(BF)
    return {"blk": blk}


def kernel(head, relation_ids, tail, relation_matrices):
    head = np.ascontiguousarray(np.asarray(head), dtype=np.float32)
    tail = np.ascontiguousarray(np.asarray(tail), dtype=np.float32)
    mats = np.ascontiguousarray(np.asarray(relation_matrices), dtype=np.float32)
    ids = np.asarray(relation_ids).astype(np.int64)
    B, D = head.shape
    R = mats.shape[0]
    assert D == DIM and R == 30

    n_blocks, blocks = _plan(ids, R)
    # [R, P, 2*DIM] bf16: mstack[r, p, c*256+j] = M_r[c*128+p, j]
    mstack = np.ascontiguousarray(
        mats.reshape(R, 2, P, DIM).transpose(0, 2, 1, 3).reshape(R, P, 2 * DIM)
    ).astype(BF)

    in_maps = [
        _core_inputs(head, tail, mstack, blocks, k, n_blocks) for k in range(NCORES)
    ]

    if n_blocks not in _prog_cache:
        _prog_cache[n_blocks] = _build(n_blocks)
    nc = _prog_cache[n_blocks]

    from concourse.bass_utils import run_bass_kernel_spmd

    kwargs = {}
    if TRACE:
        kwargs = dict(trace=True, trace_cores=list(range(NCORES)))
    try:
        res = run_bass_kernel_spmd(
            nc, in_maps, core_ids=list(range(NCORES)), **kwargs
        )
    except Exception:
        # a previous crashed session can leave the device wedged; one retry
        # after the error has been consumed usually succeeds
        import time as _time

        _time.sleep(2)
        res = run_bass_kernel_spmd(
            nc, in_maps, core_ids=list(range(NCORES)), **kwargs
        )
    global LAST_RESULT
    LAST_RESULT = res

    scores = np.zeros(B, np.float32)
    for k in range(NCORES):
        o = res.results[k]["out"]  # [P, 2*n_blocks]
        cblocks = blocks[k * n_blocks : (k + 1) * n_blocks]
        for u, (r, samp) in enumerate(cblocks):
            for s in range(2):
                sub = samp[s * P : (s + 1) * P]
                nb = len(sub)
                if nb:
                    scores[sub] = o[:nb, u * 2 + s]
    return scores



# revision 13
# speedup vs baseline: 1.0125x; 1.0125x over previous
"""Bilinear decoder kernel for Trainium2 (8 NeuronCores, SPMD).

Computes score[b] = head[b]^T @ relation_matrices[relation_ids[b]] @ tail[b]
for b in [0, 4096).

Strategy v4 (relation-grouped subtiles, deduped matrices, ACT offload):
  Host: chunk each relation's samples into segments of <=256 rows. Segments
  with >128 rows are "pairs" (full 128-subtile + remainder subtile of
  <=cap rows sharing one matrix); <=128-row segments are "singles". One
  SPMD program with G1 single slots + G2 pair slots (light cores get
  zero-filled slots).
  Bundles (bf16): sg [128, G1*1024] (per single: mat 512 | h 256 | t 256);
  per pair [128, 1280] = mat | h0 | t0 | t1; remh [128, G2*2*cap] holds
  the remainder subtiles' transposed heads (compacted to cap cols/chunk).
  DMA: sync ring FIFO [sg, remh, P1], scalar [P0, P2] — the small singles
  bundle lands first so compute starts ~0.5us earlier; rings carry equal
  bytes. Compute order: singles, P0, P1, P2 (arrival order).
  Device per pair: 4 bf16 matmuls into one [128,512] psum (remainder sub
  writes partitions [0,cap)); one joint DVE tensor_tensor [128,512]
  (psum * t0|t1 -> bf16 scr); reduces split across engines: the singles'
  and first pair's row-sums run on the otherwise-idle Scalar engine
  (activation Copy + accum_out, table load hidden behind a warmup
  activation issued during the DMA window), later pairs' segmented
  reduce_sum stays on DVE. Rows >= nb of a remainder subtile may hold
  NaN (uninitialized psum cols * zero tail) — per-row reduction keeps
  them confined to out rows the host never reads.
  Host: scatter scores back through the per-slot sample maps.

bf16 inputs, fp32 PSUM accumulate + fp32 reduce accumulators.

Notes from HW measurement (NTFF traces):
  - exec window = [first MEMSET .. last instruction]; fixed ~8.45us NEFF
    teardown (253 semaphore clears) + ~0.75us lead-in are unavoidable.
  - tensor_tensor_reduce with a PSUM in0 crashes the DVE
    (NRT_EXEC_UNIT_UNRECOVERABLE) in both real-out and broadcast-dummy
    forms; qr.py's working TTR reads SBUF only. Hence TT+reduce.
  - DMA: HWDGE issue ~0.65us + DGE delay ~0.65us + ~0.9us completion
    receipt; concurrent streams on the two rings share ~350-400GB/s.
"""

import numpy as np
import ml_dtypes

P = 128
DIM = 256
NCORES = 8
BF = ml_dtypes.bfloat16

# ACT offload is a net loss: any use of the Scalar engine's activation
# path downclocks the WHOLE chip ~20% (teardown sweep 8.45us -> 10.1us,
# memsets, matmuls, DVE all slower) — power/clock domain sharing. This
# also explains the prior session's unexplained ACT-reduce regression.
USE_ACT = False

_prog_cache = {}

# test-harness knobs: set TRACE=True before calling kernel() to capture an
# NTFF profile; the BassKernelResults lands in LAST_RESULT.
TRACE = False
LAST_RESULT = None

PW = 5 * DIM  # pair bundle cols: mat 512 | h0 256 | t0 256 | t1 256
SW = 4 * DIM  # single cols: mat 512 | h 256 | t 256


def _build(G2, G1, cap, use_act):
    import concourse.bacc as bacc
    import concourse.mybir as mybir
    import concourse.tile as tile

    f32 = mybir.dt.float32
    bf16 = mybir.dt.bfloat16
    SUBS = G1 + 2 * G2

    nc = bacc.Bacc("TRN2", target_bir_lowering=False)
    pair_in = [
        nc.dram_tensor(f"p{g}", [P, PW], bf16, kind="ExternalInput") for g in range(G2)
    ]
    # sgx = [remainder heads (G2*2*cap) | singles (G1*SW)] — one DMA; a
    # separate small remh DMA mid-FIFO cost ~1us of inter-DMA bubbles and
    # starved the DVE chain (HW-measured).
    SGW = G2 * 2 * cap + G1 * SW
    sg_in = nc.dram_tensor("sgx", [P, SGW], bf16, kind="ExternalInput")
    out = nc.dram_tensor("out", [P, SUBS], f32, kind="ExternalOutput")

    with tile.TileContext(nc) as tc:
        with (
            tc.tile_pool(name="blk", bufs=G2 + 2) as blk_pool,
            tc.tile_pool(name="scr", bufs=2) as scr_pool,
            tc.tile_pool(name="o", bufs=1) as o_pool,
            tc.tile_pool(name="psum", bufs=6, space="PSUM") as psum_pool,
        ):
            # DMA schedule (FIFO per HWDGE ring, roughly equal bytes):
            #   sync:   sg (small, lands first -> compute starts early),
            #           remh, P1
            #   scalar: P0, P2
            # Compute order singles, P0, P1, P2 matches arrival order.
            sgx_tile = blk_pool.tile([P, SGW], bf16, tag="blk")
            nc.scalar.dma_start(out=sgx_tile[:], in_=sg_in[:, :])
            RB = G2 * 2 * cap  # singles base col within sgx
            ptiles = [None] * G2
            ring = {0: nc.sync, 1: nc.scalar, 2: nc.sync}
            for g in range(G2):
                t = blk_pool.tile([P, PW], bf16, tag="blk")
                ring.get(g, nc.sync if g % 2 else nc.scalar).dma_start(
                    out=t[:], in_=pair_in[g][:, :]
                )
                ptiles[g] = t

            out_sb = o_pool.tile([P, SUBS], f32)

            if use_act:
                # warmup activation: forces the ACT function-table load to
                # happen here, hidden under the input DMA streams, instead
                # of in front of the first real reduce.
                warm = o_pool.tile([P, 1], f32)
                warm2 = o_pool.tile([P, 1], f32)
                zero = nc.const_aps.aps[(f32, 0.0)]
                nc.scalar.activation(
                    out=warm[:],
                    in_=zero,
                    func=mybir.ActivationFunctionType.Copy,
                    accum_out=warm2[:],
                )

            def act_reduce(in_ap, col):
                scr2 = scr_pool.tile([P, DIM], bf16, tag="ascr")
                nc.scalar.activation(
                    out=scr2[:],
                    in_=in_ap,
                    func=mybir.ActivationFunctionType.Copy,
                    accum_out=out_sb[:, col : col + 1],
                )

            singles_after = True
            for q in range(G1 if not singles_after else 0):
                base = RB + q * SW
                ps = psum_pool.tile([P, DIM], f32, tag="ps")
                for c in range(2):
                    nc.tensor.matmul(
                        out=ps[:],
                        lhsT=sgx_tile[
                            :, base + 2 * DIM + c * P : base + 2 * DIM + (c + 1) * P
                        ],
                        rhs=sgx_tile[:, base + c * DIM : base + (c + 1) * DIM],
                        start=(c == 0),
                        stop=(c == 1),
                    )
                scr = scr_pool.tile([P, DIM], bf16, tag="scr")
                nc.vector.tensor_tensor(
                    out=scr[:],
                    in0=ps[:],
                    in1=sgx_tile[:, base + 3 * DIM : base + 4 * DIM],
                    op=mybir.AluOpType.mult,
                )
                if use_act:
                    act_reduce(scr[:], q)
                else:
                    nc.vector.reduce_sum(
                        out=out_sb[:, q : q + 1], in_=scr[:], axis=mybir.AxisListType.X
                    )

            for g in range(G2):
                tl = ptiles[g]
                col = G1 + 2 * g
                # full subtile
                ps0 = psum_pool.tile([P, DIM], f32, tag="ps")
                for c in range(2):
                    nc.tensor.matmul(
                        out=ps0[:],
                        lhsT=tl[:, 2 * DIM + c * P : 2 * DIM + (c + 1) * P],
                        rhs=tl[:, c * DIM : (c + 1) * DIM],
                        start=(c == 0),
                        stop=(c == 1),
                    )
                scr = scr_pool.tile([P, DIM], bf16, tag="scr")
                nc.vector.tensor_tensor(
                    out=scr[:],
                    in0=ps0[:],
                    in1=tl[:, 3 * DIM : 4 * DIM],
                    op=mybir.AluOpType.mult,
                )
                nc.vector.reduce_sum(
                    out=out_sb[:, col : col + 1], in_=scr[:], axis=mybir.AxisListType.X
                )
                # remainder subtile: partitions [0,cap)
                ps1 = psum_pool.tile([P, DIM], f32, tag="ps")
                for c in range(2):
                    nc.tensor.matmul(
                        out=ps1[0:cap, :],
                        lhsT=sgx_tile[:, g * 2 * cap + c * cap : g * 2 * cap + (c + 1) * cap],
                        rhs=tl[:, c * DIM : (c + 1) * DIM],
                        start=(c == 0),
                        stop=(c == 1),
                    )
                scr1 = scr_pool.tile([P, DIM], bf16, tag="scr")
                nc.vector.tensor_tensor(
                    out=scr1[0:cap, :],
                    in0=ps1[0:cap, :],
                    in1=tl[0:cap, 4 * DIM : 5 * DIM],
                    op=mybir.AluOpType.mult,
                )
                nc.vector.reduce_sum(
                    out=out_sb[0:cap, col + 1 : col + 2],
                    in_=scr1[0:cap, :],
                    axis=mybir.AxisListType.X,
                )

            for q in range(G1):
                base = RB + q * SW
                ps = psum_pool.tile([P, DIM], f32, tag="ps")
                for c in range(2):
                    nc.tensor.matmul(
                        out=ps[:],
                        lhsT=sgx_tile[
                            :, base + 2 * DIM + c * P : base + 2 * DIM + (c + 1) * P
                        ],
                        rhs=sgx_tile[:, base + c * DIM : base + (c + 1) * DIM],
                        start=(c == 0),
                        stop=(c == 1),
                    )
                scr = scr_pool.tile([P, DIM], bf16, tag="scr")
                nc.vector.tensor_tensor(
                    out=scr[:],
                    in0=ps[:],
                    in1=sgx_tile[:, base + 3 * DIM : base + 4 * DIM],
                    op=mybir.AluOpType.mult,
                )
                nc.vector.reduce_sum(
                    out=out_sb[:, q : q + 1], in_=scr[:], axis=mybir.AxisListType.X
                )

            nc.sync.dma_start(out=out[:, :], in_=out_sb[:])

    nc.compile()
    return nc


def _plan(ids, R):
    """Chunk each relation into <=256-sample segments: pairs (>128 rows)
    and singles (<=128). Round-robin to cores, padded with None to uniform
    (G2, G1). cap = max remainder rows, rounded up to a multiple of 8."""
    pairs, singles = [], []
    for r in range(R):
        idxs = np.nonzero(ids == r)[0]
        for s in range(0, len(idxs), 2 * P):
            seg = idxs[s : s + 2 * P]
            (pairs if len(seg) > P else singles).append((r, seg))
    G2 = -(-len(pairs) // NCORES) if pairs else 0
    G1 = -(-len(singles) // NCORES) if singles else 0
    cap = 8
    for r, seg in pairs:
        cap = max(cap, len(seg) - P)
    cap = (cap + 7) // 8 * 8
    cores = []
    for k in range(NCORES):
        pk = pairs[k * G2 : (k + 1) * G2] if G2 else []
        sk = singles[k * G1 : (k + 1) * G1] if G1 else []
        pk += [None] * (G2 - len(pk))
        sk += [None] * (G1 - len(sk))
        cores.append((pk, sk))
    return G2, G1, cap, cores


def _core_inputs(head, tail, mstack, pk, sk, G2, G1, cap):
    inp = {}
    RB = G2 * 2 * cap
    sgx = np.zeros((P, RB + G1 * SW), BF)
    remh = sgx[:, :RB]
    for g in range(G2):
        blk = np.zeros((P, PW), BF)
        if pk[g] is not None:
            r, seg = pk[g]
            blk[:, 0 : 2 * DIM] = mstack[r]
            s0, s1 = seg[:P], seg[P:]
            ht = head[s0].astype(BF).T  # [DIM, 128]
            blk[:, 2 * DIM : 2 * DIM + P] = ht[:P, :]
            blk[:, 2 * DIM + P : 3 * DIM] = ht[P:, :]
            blk[:, 3 * DIM : 4 * DIM] = tail[s0].astype(BF)
            nb = len(s1)
            blk[:nb, 4 * DIM : 5 * DIM] = tail[s1].astype(BF)
            ht1 = head[s1].astype(BF).T  # [DIM, nb]
            remh[:, g * 2 * cap : g * 2 * cap + nb] = ht1[:P, :]
            remh[:, g * 2 * cap + cap : g * 2 * cap + cap + nb] = ht1[P:, :]
        inp[f"p{g}"] = blk
    for q in range(G1):
        if sk[q] is not None:
            r, seg = sk[q]
            base = RB + q * SW
            sgx[:, base : base + 2 * DIM] = mstack[r]
            nb = len(seg)
            ht = head[seg].astype(BF).T
            sgx[:, base + 2 * DIM : base + 2 * DIM + nb] = ht[:P, :]
            sgx[:, base + 2 * DIM + P : base + 2 * DIM + P + nb] = ht[P:, :]
            sgx[:nb, base + 3 * DIM : base + 4 * DIM] = tail[seg].astype(BF)
    inp["sgx"] = sgx
    return inp


def kernel(head, relation_ids, tail, relation_matrices):
    head = np.ascontiguousarray(np.asarray(head), dtype=np.float32)
    tail = np.ascontiguousarray(np.asarray(tail), dtype=np.float32)
    mats = np.ascontiguousarray(np.asarray(relation_matrices), dtype=np.float32)
    ids = np.asarray(relation_ids).astype(np.int64)
    B, D = head.shape
    R = mats.shape[0]
    assert D == DIM

    G2, G1, cap, cores = _plan(ids, R)
    # [R, P, 2*DIM] bf16: mstack[r, p, c*256+j] = M_r[c*128+p, j]
    mstack = np.ascontiguousarray(
        mats.reshape(R, 2, P, DIM).transpose(0, 2, 1, 3).reshape(R, P, 2 * DIM)
    ).astype(BF)

    in_maps = [
        _core_inputs(head, tail, mstack, pk, sk, G2, G1, cap) for pk, sk in cores
    ]

    key = (G2, G1, cap, USE_ACT)
    if key not in _prog_cache:
        _prog_cache[key] = _build(G2, G1, cap, USE_ACT)
    nc = _prog_cache[key]

    from concourse.bass_utils import run_bass_kernel_spmd

    kwargs = {}
    if TRACE:
        kwargs = dict(trace=True, trace_cores=list(range(NCORES)))
    try:
        res = run_bass_kernel_spmd(
            nc, in_maps, core_ids=list(range(NCORES)), **kwargs
        )
    except Exception:
        # a previous crashed session can leave the device wedged; one retry
        # after the error has been consumed usually succeeds
        import time as _time

        _time.sleep(2)
        res = run_bass_kernel_spmd(
            nc, in_maps, core_ids=list(range(NCORES)), **kwargs
        )
    global LAST_RESULT
    LAST_RESULT = res

    scores = np.zeros(B, np.float32)
    for k in range(NCORES):
        o = res.results[k]["out"]  # [P, SUBS]
        pk, sk = cores[k]
        for q in range(G1):
            if sk[q] is None:
                continue
            r, seg = sk[q]
            scores[seg] = o[: len(seg), q]
        for g in range(G2):
            if pk[g] is None:
                continue
            r, seg = pk[g]
            s0, s1 = seg[:P], seg[P:]
            scores[s0] = o[:P, G1 + 2 * g]
            scores[s1] = o[: len(s1), G1 + 2 * g + 1]
    return scores


# revision 15
# speedup vs baseline: 1.0515x; 1.0385x over previous
"""Bilinear decoder kernel for Trainium2 (8 NeuronCores, SPMD).

Computes score[b] = head[b]^T @ relation_matrices[relation_ids[b]] @ tail[b]
for b in [0, 4096).

Strategy (relation-grouped subtiles, deduped matrices, per-subtile DVE):
  Host: chunk each relation's samples into segments of <=256 rows. Segments
  with >128 rows are "pairs" (one full 128-row subtile + a remainder
  subtile of <=cap rows, sharing one matrix); <=128-row segments are
  "singles". One SPMD program with G2 pair slots + G1 single slots
  (light cores get zero-filled slots; program uniform across cores).
  Bundles (bf16):
    p0  [128, G2*2*cap + 1280]: ALL pairs' remainder heads (compacted to
        cap cols per contraction chunk) | mat 512 | h0 256 | t0 256 | t1 256
    p1+ [128, 1280]: mat | h0 | t0 | t1
    sg  [128, G1*1024]: per single: mat | h | t
  DMA (HWDGE FIFO per ring): sync [p0, p2, ...], scalar [p1, sg]. The
  first two pair bundles stream concurrently on the two rings and land
  first; the remainder heads ride p0 (earliest) so every pair's
  remainder matmul has its lhsT on time; the singles bundle (needed
  last) queues behind p1.
  Device per subtile: 2 accumulated bf16 matmuls
  psum[128b, 256j] += h_c[128i, nb].T @ M[128i, 256j] (remainder subs
  write partitions [0,cap) only), then per-subtile DVE tensor_tensor
  (psum * tail -> bf16 scr) + reduce_sum -> out_sb[:, sub]. Per-subtile
  granularity (vs pair-fused [128,512] DVE ops) absorbs DMA-arrival
  jitter measurably better on HW. Singles are computed last: their
  short DVE ops form the tail before the single output DMA (sync ring).
  Host: scatter scores back through the per-slot sample maps.

bf16 inputs, fp32 PSUM accumulate + fp32 reduce: absmax-relative error
3.0e-3 on the full problem (gate 2e-2).

HW-measured notes (NTFF traces; exec window = [first MEMSET .. last
instruction], includes a fixed ~8.45us NEFF teardown sweep of 253
semaphore clears + ~0.75us lead-in):
  - tensor_tensor_reduce with PSUM in0 crashes the DVE
    (NRT_EXEC_UNIT_UNRECOVERABLE) in both real-out and broadcast-dummy
    forms; qr.py's working TTR reads SBUF only. TT+reduce it is.
  - ANY use of the Scalar engine's activation path (even one warmup op)
    downclocks the whole chip ~20% (teardown 8.45us -> 10.1us, matmuls
    394 -> 474ns): power/clock-domain sharing. Never use ACT compute.
    This also explains the prior session's unexplained ACT-reduce loss.
  - A small separate remainder-heads DMA mid-FIFO adds ~1us of
    inter-DMA bubbles and starves the DVE chain; ride it on p0 instead.
  - HWDGE: issue ~0.65us + DGE delay ~0.65us + ~0.9us completion
    receipt; the two rings share ~350-400GB/s of HBM stream.
"""

import numpy as np
import ml_dtypes

P = 128
DIM = 256
NCORES = 8
BF = ml_dtypes.bfloat16

_prog_cache = {}

# test-harness knobs: set TRACE=True before calling kernel() to capture an
# NTFF profile; the BassKernelResults lands in LAST_RESULT.
TRACE = False
LAST_RESULT = None

PW = 5 * DIM  # pair payload cols: mat 512 | h0 256 | t0 256 | t1 256
SW = 4 * DIM  # single cols: mat 512 | h 256 | t 256


def _build(G2, G1, cap):
    import concourse.bacc as bacc
    import concourse.mybir as mybir
    import concourse.tile as tile

    f32 = mybir.dt.float32
    bf16 = mybir.dt.bfloat16
    SUBS = G1 + 2 * G2
    RB = G2 * 2 * cap  # remainder-heads block cols (prefix of p0)

    nc = bacc.Bacc("TRN2", target_bir_lowering=False)
    rh = 1 if G2 > 1 else 0  # which pair bundle carries the remh prefix
    pair_in = [
        nc.dram_tensor(
            f"p{g}", [P, (RB if g == rh else 0) + PW], bf16, kind="ExternalInput"
        )
        for g in range(G2)
    ]
    sg_in = (
        nc.dram_tensor("sg", [P, G1 * SW], bf16, kind="ExternalInput") if G1 else None
    )
    out = nc.dram_tensor("out", [P, SUBS], f32, kind="ExternalOutput")

    with tile.TileContext(nc) as tc:
        with (
            tc.tile_pool(name="blk", bufs=G2 + 1) as blk_pool,
            tc.tile_pool(name="scr", bufs=2) as scr_pool,
            tc.tile_pool(name="o", bufs=1) as o_pool,
            tc.tile_pool(name="psum", bufs=4, space="PSUM") as psum_pool,
        ):
            ptiles = []
            for g in range(G2):
                t = blk_pool.tile([P, (RB if g == rh else 0) + PW], bf16, tag="blk")
                eng = nc.sync if g % 2 == 0 else nc.scalar
                eng.dma_start(out=t[:], in_=pair_in[g][:, :])
                ptiles.append(t)
            sg_tile = None
            if G1:
                sg_tile = blk_pool.tile([P, G1 * SW], bf16, tag="blk")
                eng = nc.sync if G2 % 2 == 0 else nc.scalar
                eng.dma_start(out=sg_tile[:], in_=sg_in[:, :])

            out_sb = o_pool.tile([P, SUBS], f32)

            for g in range(G2):
                tl = ptiles[g]
                off = RB if g == rh else 0
                col = 2 * g
                ps = psum_pool.tile([P, 2 * DIM], f32, tag="psp")
                # full subtile -> psum cols [0,256)
                for c in range(2):
                    nc.tensor.matmul(
                        out=ps[:, 0:DIM],
                        lhsT=tl[:, off + 2 * DIM + c * P : off + 2 * DIM + (c + 1) * P],
                        rhs=tl[:, off + c * DIM : off + (c + 1) * DIM],
                        start=(c == 0),
                        stop=(c == 1),
                    )
                # remainder subtile -> psum cols [256,512), partitions [0,cap)
                for c in range(2):
                    nc.tensor.matmul(
                        out=ps[0:cap, DIM : 2 * DIM],
                        lhsT=ptiles[rh][
                            :, g * 2 * cap + c * cap : g * 2 * cap + (c + 1) * cap
                        ],
                        rhs=tl[:, off + c * DIM : off + (c + 1) * DIM],
                        start=(c == 0),
                        stop=(c == 1),
                    )
                # one fused multiply over both subtiles (t0|t1 adjacent)
                scr = scr_pool.tile([P, 2 * DIM], bf16, tag="scr")
                nc.vector.tensor_tensor(
                    out=scr[:],
                    in0=ps[:],
                    in1=tl[:, off + 3 * DIM : off + 5 * DIM],
                    op=mybir.AluOpType.mult,
                )
                nc.vector.reduce_sum(
                    out=out_sb[:, col : col + 1],
                    in_=scr[:, 0:DIM],
                    axis=mybir.AxisListType.X,
                )
                nc.vector.reduce_sum(
                    out=out_sb[0:cap, col + 1 : col + 2],
                    in_=scr[0:cap, DIM : 2 * DIM],
                    axis=mybir.AxisListType.X,
                )

            for q in range(G1):
                base = q * SW
                ps = psum_pool.tile([P, DIM], f32, tag="pss")
                for c in range(2):
                    nc.tensor.matmul(
                        out=ps[:],
                        lhsT=sg_tile[
                            :, base + 2 * DIM + c * P : base + 2 * DIM + (c + 1) * P
                        ],
                        rhs=sg_tile[:, base + c * DIM : base + (c + 1) * DIM],
                        start=(c == 0),
                        stop=(c == 1),
                    )
                scr = scr_pool.tile([P, DIM], bf16, tag="scr")
                nc.vector.tensor_tensor(
                    out=scr[:],
                    in0=ps[:],
                    in1=sg_tile[:, base + 3 * DIM : base + 4 * DIM],
                    op=mybir.AluOpType.mult,
                )
                nc.vector.reduce_sum(
                    out=out_sb[:, 2 * G2 + q : 2 * G2 + q + 1],
                    in_=scr[:],
                    axis=mybir.AxisListType.X,
                )

            nc.sync.dma_start(out=out[:, :], in_=out_sb[:])

    nc.compile()
    return nc


def _plan(ids, R):
    """Chunk each relation into <=256-sample segments: pairs (>128 rows)
    and singles (<=128). Round-robin to cores, padded with None to uniform
    (G2, G1). cap = max remainder rows, rounded up to a multiple of 8."""
    pairs, singles = [], []
    for r in range(R):
        idxs = np.nonzero(ids == r)[0]
        for s in range(0, len(idxs), 2 * P):
            seg = idxs[s : s + 2 * P]
            (pairs if len(seg) > P else singles).append((r, seg))
    if not pairs and not singles:
        singles.append((0, np.empty(0, np.int64)))
    G2 = -(-len(pairs) // NCORES) if pairs else 0
    G1 = -(-len(singles) // NCORES) if singles else 0
    cap = 8
    for r, seg in pairs:
        cap = max(cap, len(seg) - P)
    cap = (cap + 7) // 8 * 8
    cores = []
    for k in range(NCORES):
        pk = pairs[k * G2 : (k + 1) * G2] if G2 else []
        sk = singles[k * G1 : (k + 1) * G1] if G1 else []
        pk += [None] * (G2 - len(pk))
        sk += [None] * (G1 - len(sk))
        cores.append((pk, sk))
    return G2, G1, cap, cores


def _core_inputs(head, tail, mstack, pk, sk, G2, G1, cap):
    inp = {}
    RB = G2 * 2 * cap
    rh = 1 if G2 > 1 else 0
    for g in range(G2):
        off = RB if g == rh else 0
        blk = np.zeros((P, off + PW), BF)
        if pk[g] is not None:
            r, seg = pk[g]
            blk[:, off : off + 2 * DIM] = mstack[r]
            s0, s1 = seg[:P], seg[P:]
            ht = head[s0].astype(BF).T  # [DIM, 128]
            blk[:, off + 2 * DIM : off + 2 * DIM + P] = ht[:P, :]
            blk[:, off + 2 * DIM + P : off + 3 * DIM] = ht[P:, :]
            blk[:, off + 3 * DIM : off + 4 * DIM] = tail[s0].astype(BF)
            nb = len(s1)
            blk[:nb, off + 4 * DIM : off + 5 * DIM] = tail[s1].astype(BF)
        inp[f"p{g}"] = blk
    if G2:
        # remainder heads ride p{rh}'s prefix
        p0 = inp[f"p{rh}"]
        for g in range(G2):
            if pk[g] is None:
                continue
            r, seg = pk[g]
            s1 = seg[P:]
            nb = len(s1)
            if nb:
                ht1 = head[s1].astype(BF).T  # [DIM, nb]
                p0[:, g * 2 * cap : g * 2 * cap + nb] = ht1[:P, :]
                p0[:, g * 2 * cap + cap : g * 2 * cap + cap + nb] = ht1[P:, :]
    if G1:
        sg = np.zeros((P, G1 * SW), BF)
        for q in range(G1):
            if sk[q] is not None:
                r, seg = sk[q]
                base = q * SW
                sg[:, base : base + 2 * DIM] = mstack[r]
                nb = len(seg)
                ht = head[seg].astype(BF).T
                sg[:, base + 2 * DIM : base + 2 * DIM + nb] = ht[:P, :]
                sg[:, base + 2 * DIM + P : base + 2 * DIM + P + nb] = ht[P:, :]
                sg[:nb, base + 3 * DIM : base + 4 * DIM] = tail[seg].astype(BF)
        inp["sg"] = sg
    return inp


def kernel(head, relation_ids, tail, relation_matrices):
    head = np.ascontiguousarray(np.asarray(head), dtype=np.float32)
    tail = np.ascontiguousarray(np.asarray(tail), dtype=np.float32)
    mats = np.ascontiguousarray(np.asarray(relation_matrices), dtype=np.float32)
    ids = np.asarray(relation_ids).astype(np.int64)
    B, D = head.shape
    R = mats.shape[0]
    assert D == DIM

    G2, G1, cap, cores = _plan(ids, R)
    # [R, P, 2*DIM] bf16: mstack[r, p, c*256+j] = M_r[c*128+p, j]
    mstack = np.ascontiguousarray(
        mats.reshape(R, 2, P, DIM).transpose(0, 2, 1, 3).reshape(R, P, 2 * DIM)
    ).astype(BF)

    in_maps = [
        _core_inputs(head, tail, mstack, pk, sk, G2, G1, cap) for pk, sk in cores
    ]

    key = (G2, G1, cap)
    if key not in _prog_cache:
        _prog_cache[key] = _build(G2, G1, cap)
    nc = _prog_cache[key]

    from concourse.bass_utils import run_bass_kernel_spmd

    kwargs = {}
    if TRACE:
        kwargs = dict(trace=True, trace_cores=list(range(NCORES)))
    try:
        res = run_bass_kernel_spmd(
            nc, in_maps, core_ids=list(range(NCORES)), **kwargs
        )
    except Exception:
        # a previous crashed session can leave the device wedged; one retry
        # after the error has been consumed usually succeeds
        import time as _time

        _time.sleep(2)
        res = run_bass_kernel_spmd(
            nc, in_maps, core_ids=list(range(NCORES)), **kwargs
        )
    global LAST_RESULT
    LAST_RESULT = res

    scores = np.zeros(B, np.float32)
    for k in range(NCORES):
        o = res.results[k]["out"]  # [P, SUBS]
        pk, sk = cores[k]
        for g in range(G2):
            if pk[g] is None:
                continue
            r, seg = pk[g]
            s0, s1 = seg[:P], seg[P:]
            scores[s0] = o[:P, 2 * g]
            scores[s1] = o[: len(s1), 2 * g + 1]
        for q in range(G1):
            if sk[q] is None:
                continue
            r, seg = sk[q]
            scores[seg] = o[: len(seg), 2 * G2 + q]
    return scores
